# revision 12
# baseline (speedup 1.0000x reference)
"""AtomEncoder Trainium2 kernel: embeddings + residual MLP + bond aggregation.

Sharding: data-parallel over batch across 8 NeuronCores (16 batches/core).
Per core (b-major token order, t = b_local*192 + l, 3072 tokens):
  - embeddings via one-hot matmul against a combined table
    [E_elem(100); E_charge(13); E_aroma(2); E_seg(30); pe(192)] (337 rows,
    zero-padded to 384, bf16, 3 K=128 tiles). One-hot rows for the real
    tables are built on device with is_equal against iota columns; the
    positional-encoding block of the one-hot is a compile-time constant.
  - MLP GEMMs in bf16 with transposed activations [dim, tokens], tokens
    chunked 512. Biases: b1/b3 fused into the relu (alternating ACT/DVE),
    b2/b4 fused into the residual add, b5 via a K=1 ones-row matmul.
  - bond aggregation as agg = A_T.T @ msg on TensorE, where
    A_T[src,dst] = #{m: bond[dst,m]==src, src!=dst} is built with
    is_equal + segmented reduce (self-bonds pre-masked to 999 on host),
    pre-built for all batches while the MLP runs. The agg matmuls
    accumulate into the same PSUM group as the natural-layout embedding
    matmuls, so PSUM = emb + pe + agg = final output.
"""
import numpy as np
import ml_dtypes

B, L, D = 128, 192, 512
H = 4 * D                      # 2048
NCORES = 8
BPC = B // NCORES              # 16 batches per core
T = BPC * L                    # 3072 tokens per core
CH = 512                       # MLP token chunk
NCH = T // CH                  # 6 chunks
NTT = T // 128                 # 24 token tiles

_BF16 = ml_dtypes.bfloat16


def _build_nc():
    import concourse.bass as bass
    import concourse.mybir as mybir
    from concourse.tile import TileContext

    f32 = mybir.dt.float32
    bf16 = mybir.dt.bfloat16
    AF = mybir.ActivationFunctionType
    OP = mybir.AluOpType
    AX = mybir.AxisListType

    nc = bass.Bass()
    dp = nc.declare_dram_parameter
    w1d = dp("w1", [D, H], bf16, isOutput=False)
    w2d = dp("w2", [H, D], bf16, isOutput=False)
    w3d = dp("w3", [D, H], bf16, isOutput=False)
    w4d = dp("w4", [H, D], bf16, isOutput=False)
    w5d = dp("w5", [D, D], bf16, isOutput=False)
    ed = dp("eall", [384, D], bf16, isOutput=False)
    b0d = dp("b0", [128, T], bf16, isOutput=False)
    b1d_ = dp("bsrc1", [32, T], bf16, isOutput=False)
    oh1d = dp("oh1c", [128, T], bf16, isOutput=False)
    oh2d = dp("oh2c", [128, T], bf16, isOutput=False)
    miscd = dp("misc", [128, 44], f32, isOutput=False)
    bondd = dp("bondb", [BPC, 128, L * 6], bf16, isOutput=False)
    b5d = dp("b5r", [1, D], bf16, isOutput=False)
    outd = dp("out", [L, BPC, D], f32, isOutput=True)

    with TileContext(nc) as tc:
        with (
            tc.tile_pool(name="const", bufs=1) as cst,
            tc.tile_pool(name="abuf", bufs=1) as apl,
            tc.tile_pool(name="chunk", bufs=2) as cpl,
            tc.tile_pool(name="eqp", bufs=2) as eqp,
            tc.tile_pool(name="psA", bufs=4, space="PSUM") as psA,
            tc.tile_pool(name="psB", bufs=2, space="PSUM") as psB,
        ):
            es = cst.tile([128, 3, D], bf16)
            nc.sync.dma_start(es[:], ed.rearrange("(k p) n -> p k n", p=128))
            b0s = cst.tile([128, T], bf16)
            nc.sync.dma_start(b0s[:], b0d[:])
            b1s = cst.tile([32, T], bf16)
            nc.sync.dma_start(b1s[:], b1d_[:])
            oh1 = cst.tile([128, T], bf16)
            nc.sync.dma_start(oh1[17:128, :], oh1d[17:128, :])
            oh2 = cst.tile([128, T], bf16)
            nc.sync.dma_start(oh2[:], oh2d[:])
            misc = cst.tile([128, 44], f32)
            nc.sync.dma_start(misc[:], miscd[:])
            iot = misc[:, 0:4]
            bc1 = misc[:, 4:20]
            bc2 = misc[:, 20:24]
            bc3 = misc[:, 24:40]
            bc4 = misc[:, 40:44]
            b5t = cst.tile([1, D], bf16)
            nc.sync.dma_start(b5t[:], b5d[:])
            ones = cst.tile([1, 128], bf16)
            nc.gpsimd.memset(ones[:], 1.0)
            w1s = cst.tile([128, 4, H], bf16)
            nc.gpsimd.dma_start(w1s[:], w1d.rearrange("(k p) n -> p k n", p=128))
            w2s = cst.tile([128, 16, D], bf16)
            nc.gpsimd.dma_start(w2s[:], w2d.rearrange("(k p) n -> p k n", p=128))
            w3s = cst.tile([128, 4, H], bf16)
            nc.gpsimd.dma_start(w3s[:], w3d.rearrange("(k p) n -> p k n", p=128))
            w4s = cst.tile([128, 16, D], bf16)
            nc.gpsimd.dma_start(w4s[:], w4d.rearrange("(k p) n -> p k n", p=128))
            w5s = cst.tile([128, 4, D], bf16)
            nc.gpsimd.dma_start(w5s[:], w5d.rearrange("(k p) n -> p k n", p=128))

            # one-hot runtime rows
            oh0 = cst.tile([128, T], bf16)
            nc.vector.tensor_scalar(oh0[:], b0s[:], iot[:, 0:1], None, OP.is_equal)
            nc.vector.tensor_scalar(
                oh1[0:17, :], b1s[0:17, :], iot[0:17, 1:2], None, OP.is_equal
            )

            msga = [cst.tile([128, D], bf16, name=f"msga{i}", tag=f"msga{i}") for i in range(NTT)]

            # ---- A_T tiles for all batches (interleaved with MLP chunks below)
            A1s, A2s = [], []

            def build_A(bglob):
                bl = bglob % 2
                bbt = eqp.tile([128, L * 6], bf16, tag="bb")
                nc.gpsimd.dma_start(bbt[:], bondd[bglob])
                A1 = apl.tile([128, L], bf16, tag=f"A1_{bglob}")
                A2 = apl.tile([128, L], bf16, tag=f"A2_{bglob}")
                eqA = eqp.tile([128, L * 6], bf16, tag="eq")
                eqB = eqp.tile([128, L * 6], bf16, tag="eq")
                with nc.allow_low_precision(reason="bond counts <= 6 exact in bf16"):
                    if bl == 0:
                        nc.vector.tensor_scalar(eqA[:], bbt[:], iot[:, 0:1], None, OP.is_equal)
                        nc.vector.tensor_reduce(
                            A1[:], eqA[:].rearrange("p (d m) -> p d m", m=6), AX.X, OP.add)
                        nc.vector.tensor_scalar(
                            eqB[0:64, :], bbt[0:64, :], iot[0:64, 1:2], None, OP.is_equal)
                        nc.vector.tensor_reduce(
                            A2[0:64, :], eqB[0:64, :].rearrange("p (d m) -> p d m", m=6),
                            AX.X, OP.add)
                    else:
                        nc.vector.tensor_scalar(
                            eqA[64:128, :], bbt[64:128, :], iot[64:128, 2:3], None, OP.is_equal)
                        nc.vector.tensor_reduce(
                            A1[64:128, :], eqA[64:128, :].rearrange("p (d m) -> p d m", m=6),
                            AX.X, OP.add)
                        nc.vector.tensor_scalar(eqB[:], bbt[:], iot[:, 3:4], None, OP.is_equal)
                        nc.vector.tensor_reduce(
                            A2[:], eqB[:].rearrange("p (d m) -> p d m", m=6), AX.X, OP.add)
                A1s.append(A1)
                A2s.append(A2)

            for c in range(NCH):
                # spread A-builds across chunks so DVE does them during the MLP
                for bglob in range(len(A1s), min((c + 1) * 3, BPC)):
                    build_A(bglob)
                tok = slice(c * CH, (c + 1) * CH)
                # ---- emb_T -> xt
                xt = [cpl.tile([128, CH], bf16, name=f"xt{k}_{c}", tag=f"xt{k}") for k in range(4)]
                for m in range(4):
                    ps = psA.tile([128, CH], f32, tag="g")
                    ms = slice(m * 128, (m + 1) * 128)
                    nc.tensor.matmul(ps[:], es[:, 0, ms], oh0[:, tok], start=True, stop=False)
                    nc.tensor.matmul(ps[:], es[:, 1, ms], oh1[:, tok], start=False, stop=False)
                    nc.tensor.matmul(ps[:], es[:, 2, ms], oh2[:, tok], start=False, stop=True)
                    nc.scalar.activation(xt[m][:], ps[:], AF.Copy)
                # ---- GEMM1 + relu -> h (alternate ACT / DVE)
                h = [cpl.tile([128, CH], bf16, name=f"h{k}_{c}", tag=f"h{k}", bufs=1) for k in range(16)]
                for m in range(16):
                    ps = psA.tile([128, CH], f32, tag="g")
                    ms = slice(m * 128, (m + 1) * 128)
                    for k in range(4):
                        nc.tensor.matmul(ps[:], w1s[:, k, ms], xt[k][:],
                                         start=(k == 0), stop=(k == 3))
                    nc.scalar.activation(h[m][:], ps[:], AF.Relu, bias=bc1[:, m:m + 1])
                # ---- GEMM2 + residual -> x1
                x1 = [cpl.tile([128, CH], bf16, name=f"x1{k}_{c}", tag=f"x1{k}") for k in range(4)]
                for m in range(4):
                    ps = psA.tile([128, CH], f32, tag="g")
                    ms = slice(m * 128, (m + 1) * 128)
                    for k in range(16):
                        nc.tensor.matmul(ps[:], w2s[:, k, ms], h[k][:],
                                         start=(k == 0), stop=(k == 15))
                    nc.vector.scalar_tensor_tensor(
                        x1[m][:], ps[:], bc2[:, m:m + 1], xt[m][:], OP.add, OP.add)
                # ---- GEMM3 + relu -> h2
                h2 = [cpl.tile([128, CH], bf16, name=f"h2{k}_{c}", tag=f"h{k}", bufs=1) for k in range(16)]
                for m in range(16):
                    ps = psA.tile([128, CH], f32, tag="g")
                    ms = slice(m * 128, (m + 1) * 128)
                    for k in range(4):
                        nc.tensor.matmul(ps[:], w3s[:, k, ms], x1[k][:],
                                         start=(k == 0), stop=(k == 3))
                    nc.scalar.activation(h2[m][:], ps[:], AF.Relu, bias=bc3[:, m:m + 1])
                # ---- GEMM4 + residual -> x2
                x2 = [cpl.tile([128, CH], bf16, name=f"x2{k}_{c}", tag=f"x2{k}", bufs=1) for k in range(4)]
                for m in range(4):
                    ps = psA.tile([128, CH], f32, tag="g")
                    ms = slice(m * 128, (m + 1) * 128)
                    for k in range(16):
                        nc.tensor.matmul(ps[:], w4s[:, k, ms], h2[k][:],
                                         start=(k == 0), stop=(k == 15))
                    nc.vector.scalar_tensor_tensor(
                        x2[m][:], ps[:], bc4[:, m:m + 1], x1[m][:], OP.add, OP.add)
                # ---- W5: msg = x2 @ W5 + b5 into persistent msg tiles
                for tt in range(4):
                    gt = c * 4 + tt           # global token tile
                    ps = psB.tile([128, D], f32, tag="p5")
                    ts_ = slice(tt * 128, (tt + 1) * 128)
                    for k in range(4):
                        nc.tensor.matmul(ps[:], x2[k][:, ts_], w5s[:, k, :],
                                         start=(k == 0), stop=False)
                    nc.tensor.matmul(ps[:], ones[:], b5t[:], start=False, stop=True)
                    nc.scalar.activation(msga[gt][:], ps[:], AF.Copy)

            # ---- per batch: (emb_nat + agg) -> out
            for bglob in range(BPC):
                bl = bglob % 2
                A1, A2 = A1s[bglob], A2s[bglob]
                ti = (bglob * L) // 128       # first global token tile of batch
                for dt in range(2):
                    sz = 128 if dt == 0 else 64
                    ds_ = slice(dt * 128, dt * 128 + sz)
                    t0 = bglob * L + dt * 128
                    ts_ = slice(t0, t0 + sz)
                    ps = psB.tile([128, D], f32, tag="po")
                    nc.tensor.matmul(ps[0:sz, :], oh0[:, ts_], es[:, 0, :], start=True, stop=False)
                    nc.tensor.matmul(ps[0:sz, :], oh1[:, ts_], es[:, 1, :], start=False, stop=False)
                    nc.tensor.matmul(ps[0:sz, :], oh2[:, ts_], es[:, 2, :], start=False, stop=False)
                    if bl == 0:
                        nc.tensor.matmul(ps[0:sz, :], A1[:, ds_], msga[ti][:],
                                         start=False, stop=False)
                        nc.tensor.matmul(ps[0:sz, :], A2[0:64, ds_], msga[ti + 1][0:64, :],
                                         start=False, stop=True)
                    else:
                        nc.tensor.matmul(ps[0:sz, :], A1[64:128, ds_], msga[ti][64:128, :],
                                         start=False, stop=False)
                        nc.tensor.matmul(ps[0:sz, :], A2[:, ds_], msga[ti + 1][:],
                                         start=False, stop=True)
                    ot = cpl.tile([128, D], f32, tag="ot")
                    nc.scalar.activation(ot[0:sz, :], ps[0:sz, :], AF.Copy)
                    nc.sync.dma_start(outd[dt * 128:dt * 128 + sz, bglob, :], ot[0:sz, :])
    return nc


def _host_prep(element, bond, aroma, charge, segment, pe,
               E_elem, E_charge, E_aroma, E_seg,
               W1, b1, W2, b2, W3, b3, W4, b4, W5, b5):
    f32 = np.float32
    el = np.asarray(element, np.int64)
    bo = np.asarray(bond, np.int64)
    ar = np.asarray(aroma, np.int64)
    chg = np.asarray(charge, np.int64)
    sg = np.asarray(segment, np.int64)
    pe = np.asarray(pe, f32).reshape(-1, D)[:L]

    eall = np.zeros((384, D), f32)
    eall[0:100] = np.asarray(E_elem, f32)
    eall[100:113] = np.asarray(E_charge, f32)
    eall[113:115] = np.asarray(E_aroma, f32)
    eall[115:145] = np.asarray(E_seg, f32)
    eall[145:337] = pe
    eall = eall.astype(_BF16)

    # constant pe one-hot blocks (table rows 145+l): k-tile1 p=17+l (l<111),
    # k-tile2 p=l-111 (l>=111)
    lmod = np.tile(np.arange(L), BPC)
    oh1c = np.zeros((128, T), _BF16)
    prow = np.arange(17, 128)
    oh1c[17:128] = (lmod[None, :] == (prow[:, None] - 17)).astype(_BF16)
    oh2c = np.zeros((128, T), _BF16)
    prow2 = np.arange(0, 81)
    oh2c[0:81] = (lmod[None, :] == (prow2[:, None] + 111)).astype(_BF16)

    io4 = np.stack([np.arange(128), np.arange(128) + 128,
                    np.arange(128) - 64, np.arange(128) + 64], 1).astype(f32)

    bom = bo.astype(f32)
    self_mask = bo == np.arange(L)[None, :, None]
    bom[self_mask] = 999.0
    bom = bom.astype(_BF16)

    shared = {
        "w1": np.asarray(W1, f32).astype(_BF16),
        "w2": np.asarray(W2, f32).astype(_BF16),
        "w3": np.asarray(W3, f32).astype(_BF16),
        "w4": np.asarray(W4, f32).astype(_BF16),
        "w5": np.asarray(W5, f32).astype(_BF16),
        "eall": eall,
        "oh1c": oh1c, "oh2c": oh2c,
        "misc": np.concatenate([
            io4,
            np.asarray(b1, f32).reshape(16, 128).T,
            np.asarray(b2, f32).reshape(4, 128).T,
            np.asarray(b3, f32).reshape(16, 128).T,
            np.asarray(b4, f32).reshape(4, 128).T,
        ], axis=1).astype(f32),
        "b5r": np.asarray(b5, f32).reshape(1, D).astype(_BF16),
    }

    in_maps = []
    for cid in range(NCORES):
        bs = slice(cid * BPC, (cid + 1) * BPC)
        elf = el[bs].reshape(T).astype(f32)
        chf = chg[bs].reshape(T).astype(f32) + 106.0
        arf = ar[bs].reshape(T).astype(f32) + 113.0
        sgf = sg[bs].reshape(T).astype(f32) + 115.0
        b0 = np.empty((128, T), _BF16)
        b0[0:100] = elf
        b0[100:113] = chf
        b0[113:115] = arf
        b0[115:128] = sgf
        bs1 = np.full((32, T), -1.0, _BF16)
        bs1[0:17] = sgf
        bondb = np.broadcast_to(
            bom[bs].reshape(BPC, 1, L * 6), (BPC, 128, L * 6)).copy()
        in_maps.append(dict(shared, b0=b0, bsrc1=bs1, bondb=bondb))
    return in_maps


_COMPILED = {}


def kernel(**inputs):
    import sys
    for p in ("/opt/trn_rl_repo", "/opt/pypackages"):
        if p not in sys.path:
            sys.path.append(p)
    _install_wait_split()
    from concourse.bass_utils import run_bass_kernel_spmd

    if "nc" not in _COMPILED:
        _COMPILED["nc"] = _build_nc()
    nc = _COMPILED["nc"]
    in_maps = _host_prep(**inputs)
    res = run_bass_kernel_spmd(nc, in_maps, list(range(NCORES)), trace=False)
    out = np.concatenate([res.results[c]["out"] for c in range(NCORES)], axis=1)
    return out.astype(np.float32)


def _install_wait_split():
    """walrus in this env accepts one sync wait per instruction; Tile can emit
    several. Split extras into single-wait NoOps at BIR-JSON level."""
    import orjson
    import concourse.bass as _bass
    if getattr(_bass.Bass, "_wait_split_installed", False):
        return
    orig = _bass.Bass.to_json_bytes

    def _split(bir):
        d = orjson.loads(bir)
        ctr = 0
        changed = False
        for fn in d.get("functions", []):
            for blk in fn.get("blocks", []):
                out = []
                for inst in blk.get("instructions") or []:
                    si = inst.get("sync_info")
                    waits = (si or {}).get("on_wait") or []
                    if len(waits) > 1:
                        changed = True
                        for w in waits[:-1]:
                            ctr += 1
                            out.append({
                                "name": f"{inst['name']}-wsplit{ctr}",
                                "opcode": "NoOp",
                                "engine": inst["engine"],
                                "ins": [], "outs": [],
                                "sync_info": {"on_wait": [w], "on_update": []},
                            })
                        si["on_wait"] = [waits[-1]]
                    out.append(inst)
                blk["instructions"] = out
        return orjson.dumps(d) if changed else bir

    def to_json_bytes(self):
        return _split(orig(self))

    _bass.Bass.to_json_bytes = to_json_bytes
    _bass.Bass._wait_split_installed = True
